# revision 1
# baseline (speedup 1.0000x reference)
"""Trainium2 Bass kernel: 2-layer LSTM (B=256, T=512, H=512) -> linear head.

Strategy:
  - Data-parallel over batch: 8 cores x 32 rows each, weights replicated,
    no inter-core communication (recurrence is per-batch-row independent).
  - Per step, per layer: gates^T computed batch-major with the h-state as the
    *stationary* matmul operand ([128 h-dims x 32 batch] per K-chunk) and the
    transposed weight matrix W^T as the *moving* operand, 4x col-tiled so all
    four 32-partition output groups of one PSUM bank accumulate concurrently.
  - Gate-dim blocks permuted to (i, f, o, g) so one sigmoid covers cols 0:384
    and one tanh covers 384:512 of the PSUM bank.
  - x_t (scalar input) and biases folded into the same PSUM accumulation with a
    K=2 rank-2 matmul: lhsT = [x_t; 1] (from a preloaded x image), rhs =
    [W_ih1; b] constant image.
  - State updates (c = f*c + i*g, h = o*tanh(c)) on VectorE as [128,128] tiles;
    h transposed back to stationary layout with 4 PE transposes + 1 copy.
"""

import os
import numpy as np

B, T, H, C = 256, 512, 512, 10
NCORES = 8
BC = B // NCORES  # 32
G4 = 4 * H  # 2048

# gate blocks reordered (i, f, o, g): sigmoid gates contiguous
_PERM = np.concatenate(
    [np.arange(0, 512), np.arange(512, 1024), np.arange(1536, 2048), np.arange(1024, 1536)]
)

_BUILD_CACHE = {}


def _build(t_steps=T):
    """Trace + schedule + compile the bass module. Returns (nc, io names)."""
    import concourse.bass as bass
    import concourse.tile as tile
    from concourse import bacc, mybir
    from contextlib import ExitStack

    f32 = mybir.dt.float32
    bf16 = mybir.dt.bfloat16
    AF = mybir.ActivationFunctionType
    assert t_steps % 4 == 0
    tq_len = t_steps // 4  # steps handled per q row-pair

    nc = bacc.Bacc("TRN2", target_bir_lowering=False, debug=False, num_devices=NCORES)

    dW1 = nc.dram_tensor("w1t", (128, 4 * G4), bf16, kind="ExternalInput").ap()
    dW2i = nc.dram_tensor("w2it", (128, 4 * G4), bf16, kind="ExternalInput").ap()
    dW2h = nc.dram_tensor("w2ht", (128, 4 * G4), bf16, kind="ExternalInput").ap()
    dWI1 = nc.dram_tensor("wi1img", (128, 512), f32, kind="ExternalInput").ap()
    dB1 = nc.dram_tensor("b1img", (128, 512), f32, kind="ExternalInput").ap()
    dXC = nc.dram_tensor("xcols", (128, t_steps), f32, kind="ExternalInput").ap()
    dXB2 = nc.dram_tensor("xb2", (128, G4), bf16, kind="ExternalInput").ap()
    dXA = nc.dram_tensor("xaug", (128, BC * tq_len), bf16, kind="ExternalInput").ap()
    dWoT = nc.dram_tensor("wot", (128, 4 * C), bf16, kind="ExternalInput").ap()
    dId = nc.dram_tensor("id32", (128, 128), bf16, kind="ExternalInput").ap()
    dY = nc.dram_tensor("y", (BC, C), f32, kind="ExternalOutput").ap()

    def w_ap(tile_ap, k, jh):
        # cols of W^T image: 2048*k + 512*gi + 128*jh + h' ; returns [128, gi=4, h'=128]
        return tile_ap.rearrange("p (k gi j h) -> p k gi j h", k=4, gi=4, j=4, h=128)[
            :, k, :, jh, :
        ]

    def xb_ap(tile_ap, q, jh):
        # [2, gi=4, h'=128] slice of the [128, 2048] (W_ih/bias) image at row pair 32q
        return tile_ap.rearrange("p (gi j h) -> p gi j h", gi=4, j=4, h=128)[
            32 * q : 32 * q + 2, :, jh, :
        ]

    with tile.TileContext(nc) as tc, ExitStack() as ctx:
        const = ctx.enter_context(tc.tile_pool(name="const", bufs=1))
        W1 = const.tile([128, 4 * G4], bf16, tag="w1")
        W2i = const.tile([128, 4 * G4], bf16, tag="w2i")
        W2h = const.tile([128, 4 * G4], bf16, tag="w2h")
        WI1 = const.tile([128, 512], f32, tag="wi1img")
        B1 = const.tile([128, 512], f32, tag="b1img")
        XC = const.tile([128, t_steps], f32, tag="xcols")
        XB2 = const.tile([128, G4], bf16, tag="xb2")
        XA = const.tile([128, BC * tq_len], bf16, tag="xa")
        WoT = const.tile([128, 4 * C], bf16, tag="wot")
        Id32 = const.tile([128, 128], bf16, tag="id32")
        Zrow = const.tile([1, 128], bf16, tag="zrow")
        nc.vector.memset(Zrow[:], 0.0)
        nc.sync.dma_start(W1[:], dW1)
        nc.sync.dma_start(W2i[:], dW2i)
        nc.sync.dma_start(W2h[:], dW2h)
        nc.sync.dma_start(WI1[:], dWI1)
        nc.sync.dma_start(B1[:], dB1)
        nc.sync.dma_start(XC[:], dXC)
        nc.sync.dma_start(XB2[:], dXB2)
        nc.sync.dma_start(XA[:], dXA)
        nc.sync.dma_start(WoT[:], dWoT)
        nc.sync.dma_start(Id32[:], dId)

        pg1p = ctx.enter_context(tc.tile_pool(name="pg1", bufs=1, space="PSUM"))
        pg2p = ctx.enter_context(tc.tile_pool(name="pg2", bufs=2, space="PSUM"))
        ptrp = ctx.enter_context(tc.tile_pool(name="ptr", bufs=2, space="PSUM"))
        poutp = ctx.enter_context(tc.tile_pool(name="pout", bufs=1, space="PSUM"))
        junkp = ctx.enter_context(tc.tile_pool(name="junk", bufs=1, space="PSUM"))

        gact = ctx.enter_context(tc.tile_pool(name="gact", bufs=3))
        statep = ctx.enter_context(tc.tile_pool(name="state", bufs=3))
        tmpp = ctx.enter_context(tc.tile_pool(name="tmp", bufs=4))
        outp = ctx.enter_context(tc.tile_pool(name="out", bufs=1))

        h1T = statep.tile([128, 128], bf16, tag="h1T")
        h2T = statep.tile([128, 128], bf16, tag="h2T")
        c1 = statep.tile([128, 128], f32, tag="c1")
        c2 = statep.tile([128, 128], f32, tag="c2")
        for st in (h1T, h2T, c1, c2):
            nc.vector.memset(st[:], 0.0)

        def epilogue_math(pg, c_prev, tagsuf):
            """activations + state update; returns (hbm, c_new)."""
            g = gact.tile([128, 512], f32, tag="g" + tagsuf)
            nc.scalar.activation(g[:], pg[:], AF.Sigmoid)
            nc.vector.tensor_scalar(
                g[:, 384:512], g[:, 384:512], 2.0, 1.0,
                mybir.AluOpType.mult, mybir.AluOpType.subtract,
            )
            i_ = g[:, 0:128]
            f_ = g[:, 128:256]
            o_ = g[:, 256:384]
            gg = g[:, 384:512]
            tig = tmpp.tile([128, 128], bf16, tag="tig" + tagsuf)
            nc.vector.tensor_mul(tig[:], i_, gg)
            tfc = tmpp.tile([128, 128], f32, tag="tfc" + tagsuf)
            nc.vector.tensor_mul(tfc[:], f_, c_prev[:])
            c_new = statep.tile([128, 128], f32, tag="c" + tagsuf)
            nc.vector.tensor_add(c_new[:], tig[:], tfc[:])
            tc_ = tmpp.tile([128, 128], bf16, tag="tc" + tagsuf)
            nc.scalar.activation(tc_[:], c_new[:], AF.Tanh)
            hbm = tmpp.tile([128, 128], bf16, tag="hbm" + tagsuf)
            nc.vector.tensor_mul(hbm[:], o_, tc_[:])
            return hbm, c_new

        def transpose_cast(hbm, tagsuf, on_act=False):
            pt = ptrp.tile([128, 128], f32, tag="pt")
            nc.tensor.matmul(pt[:], hbm[:], Id32[:], start=True, stop=True)
            hT_new = statep.tile([128, 128], bf16, tag="hT" + tagsuf)
            if on_act:
                nc.scalar.copy(hT_new[:], pt[:])
            else:
                nc.vector.tensor_copy(hT_new[:], pt[:])
            return hT_new

        pg1_bank_a = pg1p.tile([128, 512], f32, tag="pg1a")
        pg1_bank_b = pg1p.tile([128, 512], f32, tag="pg1b")
        pg1_banks = [pg1_bank_a, pg1_bank_b]
        for bank in pg1_banks:
            nc.tensor.matmul(bank[:], Zrow[:], W1[0:1, 0:512], start=True, stop=True,
                             skip_group_check=True)

        def prime_pg1(t):
            nc.vector.scalar_tensor_tensor(
                pg1_banks[t % 2][:], WI1[:], XC[:, t : t + 1], B1[:],
                mybir.AluOpType.mult, mybir.AluOpType.add,
            )

        prime_pg1(0)
        h2bm_prev = None
        for t in range(t_steps):
            q, tqi = divmod(t, tq_len)
            xa2 = XA[32 * q : 32 * q + 2, BC * tqi : BC * tqi + BC]

            # layer 1 matmuls first: they only need h1T from the previous step,
            # so they must sit ahead of anything that waits on the layer-2 chain
            pg1 = pg1_banks[t % 2]
            for k in range(4):
                for jh in range(4):
                    nc.tensor.matmul(
                        pg1[32 * jh : 32 * jh + 32, :],
                        h1T[:, 32 * k : 32 * k + 32],
                        w_ap(W1[:], k, jh),
                        start=False,
                        stop=False,
                        tile_position=(0, 32 * jh),
                        skip_group_check=True,
                    )

            if t + 1 < t_steps:
                prime_pg1(t + 1)

            # deferred transpose of h2 from the previous step, then the
            # layer-2 recurrent matmuls for this step
            if h2bm_prev is not None:
                h2T = transpose_cast(h2bm_prev, "2", on_act=True)
            pg2 = pg2p.tile([128, 512], f32, tag="pg2")
            nc.tensor.matmul(pg2[:], Zrow[:], W1[0:1, 0:512], start=True, stop=False,
                             skip_group_check=True)
            for k in range(4):
                for jh in range(4):
                    nc.tensor.matmul(
                        pg2[32 * jh : 32 * jh + 32, :],
                        h2T[:, 32 * k : 32 * k + 32],
                        w_ap(W2h[:], k, jh),
                        start=False,
                        stop=False,
                        tile_position=(0, 32 * jh),
                        skip_group_check=True,
                    )

            # HAM warm-keepers: discarded matmuls that keep the PE clock at
            # 8/8 through the layer-1 epilogue bubble
            junk = junkp.tile([32, 512], f32, tag="junk")
            for _ in range(6):
                nc.tensor.matmul(junk[:], h2T[:, 0:32], W1[:, 0:512],
                                 start=True, stop=True, skip_group_check=True)

            # layer-1 epilogue + transpose (the critical recurrence chain)
            h1bm, c1 = epilogue_math(pg1, c1, "1")
            h1T = transpose_cast(h1bm, "1")

            # layer-2 input part (needs new h1T) + bias, closes the pg2 group
            for k in range(4):
                for jh in range(4):
                    nc.tensor.matmul(
                        pg2[32 * jh : 32 * jh + 32, :],
                        h1T[:, 32 * k : 32 * k + 32],
                        w_ap(W2i[:], k, jh),
                        start=False,
                        stop=False,
                        tile_position=(0, 32 * jh),
                        skip_group_check=True,
                    )
            for jh in range(4):
                nc.tensor.matmul(
                    pg2[32 * jh : 32 * jh + 32, :], xa2, xb_ap(XB2[:], q, jh),
                    start=False, stop=(jh == 3), tile_position=(32 * q, 32 * jh),
                    skip_group_check=True,
                )

            # layer-2 epilogue; its transpose is deferred into step t+1
            h2bm_prev, c2 = epilogue_math(pg2, c2, "2")

        h2T = transpose_cast(h2bm_prev, "2", on_act=True)

        # head: y[32,10] = h2 @ W_out.T
        pout = poutp.tile([BC, C], f32, tag="pout")
        for k in range(4):
            nc.tensor.matmul(
                pout[:],
                h2T[:, 32 * k : 32 * k + 32],
                WoT[:, 10 * k : 10 * k + 10],
                start=(k == 0),
                stop=(k == 3),
            )
        ysb = outp.tile([BC, C], f32, tag="ysb")
        nc.vector.tensor_copy(ysb[:], pout[:])
        nc.sync.dma_start(dY, ysb[:])

    nc.compile()
    return nc


def _prep_consts(W_ih1, W_hh1, b_ih1, b_hh1, W_ih2, W_hh2, b_ih2, b_hh2, W_out):
    """Host-side layout transforms shared by all cores."""
    p = _PERM
    w1p = np.asarray(W_hh1, np.float32)[p].copy()  # [2048, 512]
    w2ip = np.asarray(W_ih2, np.float32)[p].copy()
    w2hp = np.asarray(W_hh2, np.float32)[p].copy()

    def wt_img(w):  # -> [128, 4*2048]
        out = np.empty((128, 4 * G4), np.float32)
        for k in range(4):
            out[:, G4 * k : G4 * (k + 1)] = w[:, 128 * k : 128 * (k + 1)].T
        return out

    xb2 = np.zeros((128, G4), np.float32)
    wih1p = np.asarray(W_ih1, np.float32)[p, 0]
    bias1p = (np.asarray(b_ih1, np.float32) + np.asarray(b_hh1, np.float32))[p]
    bias2p = (np.asarray(b_ih2, np.float32) + np.asarray(b_hh2, np.float32))[p]
    # tanh-as-sigmoid trick: pre-scale the g-gate rows by 2 everywhere
    gsl = slice(1536, 2048)
    w1p[gsl] *= 2.0
    w2ip[gsl] *= 2.0
    w2hp[gsl] *= 2.0
    wih1p[gsl] *= 2.0
    bias1p[gsl] *= 2.0
    bias2p = bias2p.copy()
    bias2p[gsl] *= 2.0
    for qq in range(4):
        xb2[32 * qq + 1] = bias2p
    # chunked-layout images for the pg1 DVE prime: [32jh+b, 128gi+h'] = val[512gi+128jh+h']
    def chunk_img(vec):
        out = np.empty((128, 512), np.float32)
        for jh in range(4):
            for gi in range(4):
                out[32 * jh : 32 * jh + 32, 128 * gi : 128 * (gi + 1)] = vec[
                    512 * gi + 128 * jh : 512 * gi + 128 * (jh + 1)
                ][None, :]
        return out

    wi1_img = chunk_img(wih1p)
    b1_img = chunk_img(bias1p)

    wot = np.empty((128, 4 * C), np.float32)
    wo = np.asarray(W_out, np.float32)
    for k in range(4):
        wot[:, C * k : C * (k + 1)] = wo[:, 128 * k : 128 * (k + 1)].T

    import ml_dtypes

    bf = ml_dtypes.bfloat16
    return {
        "w1t": wt_img(w1p).astype(bf),
        "w2it": wt_img(w2ip).astype(bf),
        "w2ht": wt_img(w2hp).astype(bf),
        "wi1img": wi1_img,
        "b1img": b1_img,
        "xb2": xb2.astype(bf),
        "wot": wot.astype(bf),
        "id32": np.eye(128, dtype=np.float32).astype(bf),
    }


def _prep_xcols(x_shard, t_steps=T):
    """[128, T] fp32: col t = x_t per partition (batch replicated per jh group)."""
    xs = np.asarray(x_shard, np.float32)[:, :t_steps]  # [32, T]
    return np.tile(xs, (4, 1)).reshape(128, t_steps)


def _prep_xaug(x_shard, t_steps=T):
    """x image [128, 32*(T/4)]: row 32q = x^T flat for its span, row 32q+1 = ones."""
    tq_len = t_steps // 4
    import ml_dtypes

    xa = np.zeros((128, BC * tq_len), ml_dtypes.bfloat16)
    xs = np.asarray(x_shard, np.float32)  # [32, T]
    for qq in range(4):
        span = xs[:, qq * tq_len : (qq + 1) * tq_len]  # [32, tq_len]
        xa[32 * qq] = span.T.reshape(-1).astype(ml_dtypes.bfloat16)  # col 32*tq + b
        xa[32 * qq + 1] = 1.0
    return xa




def _install_ntff_hook():
    """Provide antenv.axon_hooks (absent in this image) so trace=True works."""
    import sys, types
    if "antenv.axon_hooks" in sys.modules:
        return
    try:
        import antenv
        from trn_agent_boot.trn_boot import _ntff_profile_via_ctypes
    except Exception:
        return
    mod = types.ModuleType("antenv.axon_hooks")
    holder = {}
    mod.set_axon_ntff_profile_hook = lambda h: holder.__setitem__("h", h)
    mod.get_axon_ntff_profile_hook = lambda: holder.get("h")
    sys.modules["antenv.axon_hooks"] = mod
    antenv.axon_hooks = mod
    try:
        hook = _ntff_profile_via_ctypes("/opt/axon/libaxon_pjrt.so")
        if hook is not None:
            mod.set_axon_ntff_profile_hook(hook)
    except Exception:
        pass

def kernel(x, W_ih1, W_hh1, b_ih1, b_hh1, W_ih2, W_hh2, b_ih2, b_hh2, W_out, b_out):
    import sys

    for pth in ("/opt/trn_rl_repo", "/root/.axon_site/_ro/trn_rl_repo"):
        if os.path.isdir(pth) and pth not in sys.path:
            sys.path.append(pth)
    from concourse import bass_utils

    if "nc" not in _BUILD_CACHE:
        _BUILD_CACHE["nc"] = _build(T)
    nc = _BUILD_CACHE["nc"]

    consts = _prep_consts(W_ih1, W_hh1, b_ih1, b_hh1, W_ih2, W_hh2, b_ih2, b_hh2, W_out)
    in_maps = []
    for cidx in range(NCORES):
        sl = slice(BC * cidx, BC * (cidx + 1))
        xs = np.asarray(x)[sl]
        in_maps.append({**consts, "xaug": _prep_xaug(xs), "xcols": _prep_xcols(xs)})

    trace = bool(int(os.environ.get("KERNEL_TRACE", "0")))
    if trace:
        _install_ntff_hook()
    res = bass_utils.run_bass_kernel_spmd(nc, in_maps, core_ids=list(range(NCORES)), trace=trace)
    _BUILD_CACHE["last_results"] = res

    out = np.empty((B, C), np.float32)
    bo = np.asarray(b_out, np.float32)
    for cidx in range(NCORES):
        out[BC * cidx : BC * (cidx + 1)] = res.results[cidx]["y"] + bo
    return out



# revision 15
# speedup vs baseline: 1.1603x; 1.1603x over previous
"""Trainium2 Bass kernel: 2-layer LSTM (B=256, T=512, H=512) -> linear head.

Strategy:
  - Data-parallel over batch: 8 cores x 32 rows each, weights replicated,
    no inter-core communication (recurrence is per-batch-row independent).
  - Per step, per layer: gates^T computed batch-major with the h-state as the
    *stationary* matmul operand ([128 h-dims x 32 batch] per K-chunk) and the
    transposed weight matrix W^T as the *moving* operand, 4x col-tiled so all
    four 32-partition output groups of one PSUM bank accumulate concurrently.
  - Gate-dim blocks permuted to (i, f, o, g); the epilogue runs three Act
    instructions sigmoid(i,f) / tanh(g) / sigmoid(o) so the Vector c-update
    chain starts as early as possible; gates are bf16 for 2x DVE throughput.
  - Layer-2 bias injected by a K=2 start=True matmul (rows [x_t; 1] against a
    [0; b2] image) that opens the PSUM accumulation group; layer-1 bias and
    scalar-input term written by a Vector scalar_tensor_tensor prime.
  - One-step software-pipeline skew: iteration t runs L1(t), its epilogue and
    h1-transpose, then L2(t-1) matmuls + epilogue + h2-transpose, so the PE
    stream stays dense and the h1 recurrence cycle never waits on layer 2.
"""

import os
import numpy as np

B, T, H, C = 256, 512, 512, 10
NCORES = 8
BC = B // NCORES  # 32
G4 = 4 * H  # 2048

# gate blocks reordered (i, f, o, g): sigmoids contiguous, tanh block last
_PERM = np.concatenate(
    [np.arange(0, 512), np.arange(512, 1024), np.arange(1536, 2048), np.arange(1024, 1536)]
)

_BUILD_CACHE = {}


def _build(t_steps=T):
    """Trace + schedule + compile the bass module. Returns nc."""
    import concourse.bass as bass
    import concourse.tile as tile
    from concourse import bacc, mybir
    from contextlib import ExitStack

    f32 = mybir.dt.float32
    bf16 = mybir.dt.bfloat16
    AF = mybir.ActivationFunctionType
    assert t_steps % 4 == 0
    tq_len = t_steps // 4  # steps handled per q row-pair

    nc = bacc.Bacc("TRN2", target_bir_lowering=False, debug=False, num_devices=NCORES)

    dW1 = nc.dram_tensor("w1t", (128, 4 * G4), bf16, kind="ExternalInput").ap()
    dW2i = nc.dram_tensor("w2it", (128, 4 * G4), bf16, kind="ExternalInput").ap()
    dW2h = nc.dram_tensor("w2ht", (128, 4 * G4), bf16, kind="ExternalInput").ap()
    dWI1 = nc.dram_tensor("wi1img", (128, 512), f32, kind="ExternalInput").ap()
    dB1 = nc.dram_tensor("b1img", (128, 512), f32, kind="ExternalInput").ap()
    dXC = nc.dram_tensor("xcols", (128, t_steps), f32, kind="ExternalInput").ap()
    dIND4 = nc.dram_tensor("ind4", (4, 128), bf16, kind="ExternalInput").ap()
    dXB2R = nc.dram_tensor("xb2r", (4, 512), bf16, kind="ExternalInput").ap()
    dWoT = nc.dram_tensor("wot", (128, 4 * C), bf16, kind="ExternalInput").ap()
    dId = nc.dram_tensor("id32", (128, 128), bf16, kind="ExternalInput").ap()
    dY = nc.dram_tensor("y", (BC, C), f32, kind="ExternalOutput").ap()

    def w_ap(tile_ap, k, jh):
        # cols of W^T image: 2048*k + 512*gi + 128*jh + h' ; returns [128, gi=4, h'=128]
        return tile_ap.rearrange("p (k gi j h) -> p k gi j h", k=4, gi=4, j=4, h=128)[
            :, k, :, jh, :
        ]

    with tile.TileContext(nc) as tc, ExitStack() as ctx:
        const = ctx.enter_context(tc.tile_pool(name="const", bufs=1))
        W1 = const.tile([128, 4 * G4], bf16, tag="w1")
        W2i = const.tile([128, 4 * G4], bf16, tag="w2i")
        W2h = const.tile([128, 4 * G4], bf16, tag="w2h")
        WI1 = const.tile([128, 512], f32, tag="wi1img")
        B1 = const.tile([128, 512], f32, tag="b1img")
        XC = const.tile([128, t_steps], f32, tag="xcols")
        IND4 = const.tile([4, 128], bf16, tag="ind4")
        XB2R = const.tile([4, 512], bf16, tag="xb2r")
        WoT = const.tile([128, 4 * C], bf16, tag="wot")
        Id32 = const.tile([128, 128], bf16, tag="id32")
        Zrow = const.tile([1, 128], bf16, tag="zrow")
        nc.vector.memset(Zrow[:], 0.0)
        nc.sync.dma_start(W1[:], dW1)
        nc.sync.dma_start(W2i[:], dW2i)
        nc.sync.dma_start(W2h[:], dW2h)
        nc.sync.dma_start(WI1[:], dWI1)
        nc.sync.dma_start(B1[:], dB1)
        nc.sync.dma_start(XC[:], dXC)
        nc.sync.dma_start(IND4[:], dIND4)
        nc.sync.dma_start(XB2R[:], dXB2R)
        nc.sync.dma_start(WoT[:], dWoT)
        nc.sync.dma_start(Id32[:], dId)

        pg1p = ctx.enter_context(tc.tile_pool(name="pg1", bufs=1, space="PSUM"))
        pg2p = ctx.enter_context(tc.tile_pool(name="pg2", bufs=1, space="PSUM"))
        ptrp = ctx.enter_context(tc.tile_pool(name="ptr", bufs=2, space="PSUM"))
        poutp = ctx.enter_context(tc.tile_pool(name="pout", bufs=1, space="PSUM"))

        gact = ctx.enter_context(tc.tile_pool(name="gact", bufs=3))
        statep = ctx.enter_context(tc.tile_pool(name="state", bufs=3))
        tmpp = ctx.enter_context(tc.tile_pool(name="tmp", bufs=4))
        outp = ctx.enter_context(tc.tile_pool(name="out", bufs=1))

        h1T = statep.tile([128, 128], bf16, tag="hT1")
        h2T = statep.tile([128, 128], bf16, tag="hT2")
        c1 = statep.tile([128, 128], f32, tag="c1")
        c2 = statep.tile([128, 128], f32, tag="c2")
        for st in (h1T, h2T, c1, c2):
            nc.vector.memset(st[:], 0.0)

        pg1_banks = [pg1p.tile([128, 512], f32, tag="pg1a", name="pg1a"),
                     pg1p.tile([128, 512], f32, tag="pg1b", name="pg1b")]
        pg2_banks = [pg2p.tile([128, 512], f32, tag="pg2a", name="pg2a"),
                     pg2p.tile([128, 512], f32, tag="pg2b", name="pg2b")]
        # one-time has_written priming so start=False matmuls accumulate onto
        # the DVE-primed values
        for bank in pg1_banks:
            nc.tensor.matmul(bank[:], Zrow[:], W1[0:1, 0:512], start=True, stop=True,
                             skip_group_check=True)

        def prime_pg1(t):
            # pg1 <- W_ih1 * x_t + (b_ih1 + b_hh1), one DVE pass
            nc.vector.scalar_tensor_tensor(
                pg1_banks[t % 2][:], WI1[:], XC[:, t : t + 1], B1[:],
                mybir.AluOpType.mult, mybir.AluOpType.add,
            )

        def mm16(pg, hT, W, stop_last=False):
            for k in range(4):
                for jh in range(4):
                    nc.tensor.matmul(
                        pg[32 * jh : 32 * jh + 32, :],
                        hT[:, 32 * k : 32 * k + 32],
                        w_ap(W[:], k, jh),
                        start=False,
                        stop=(stop_last and k == 3 and jh == 3),
                        tile_position=(0, 32 * jh),
                        skip_group_check=True,
                    )

        def xa_open(pg):
            # opens the pg2 accumulation group with a single K=4 M=128 matmul
            # (indicator rows x bias image) so the whole-bank has_written clear
            # of start=True happens exactly once, then writes b2 everywhere
            nc.tensor.matmul(pg[:], IND4[:], XB2R[:], start=True, stop=False,
                             skip_group_check=True)

        def epilogue(pg, c_prev, tagsuf):
            """sigmoid(i,f) / tanh(g) / sigmoid(o) + state update -> (hbm, c_new)."""
            g = gact.tile([128, 512], bf16, tag="g" + tagsuf)
            nc.scalar.activation(g[:, 0:256], pg[:, 0:256], AF.Sigmoid)
            nc.scalar.activation(g[:, 384:512], pg[:, 384:512], AF.Tanh)
            nc.scalar.activation(g[:, 256:384], pg[:, 256:384], AF.Sigmoid)
            tfc = tmpp.tile([128, 128], f32, tag="tfc" + tagsuf)
            nc.vector.tensor_mul(tfc[:], g[:, 128:256], c_prev[:])
            tig = tmpp.tile([128, 128], bf16, tag="tig" + tagsuf)
            nc.vector.tensor_mul(tig[:], g[:, 0:128], g[:, 384:512])
            c_new = statep.tile([128, 128], f32, tag="c" + tagsuf)
            nc.vector.tensor_add(c_new[:], tig[:], tfc[:])
            tc_ = tmpp.tile([128, 128], bf16, tag="tc" + tagsuf)
            nc.scalar.activation(tc_[:], c_new[:], AF.Tanh)
            hbm = tmpp.tile([128, 128], bf16, tag="hbm" + tagsuf)
            nc.vector.tensor_mul(hbm[:], g[:, 256:384], tc_[:])
            return hbm, c_new

        def transpose_cast(hbm, tagsuf):
            pt = ptrp.tile([128, 128], f32, tag="pt")
            nc.tensor.matmul(pt[:], hbm[:], Id32[:], start=True, stop=True)
            hT_new = statep.tile([128, 128], bf16, tag="hT" + tagsuf)
            nc.vector.tensor_copy(hT_new[:], pt[:])
            return hT_new

        prime_pg1(0)
        xa_open(pg2_banks[0])
        for t in range(t_steps):
            # --- layer-1 for step t: matmuls, epilogue, transpose ---
            pg1 = pg1_banks[t % 2]
            mm16(pg1, h1T, W1)
            h1bm, c1 = epilogue(pg1, c1, "1")
            h1T_new = transpose_cast(h1bm, "1")

            # --- layer-2 for step t-1 (one-step pipeline skew) ---
            if t > 0:
                pg2 = pg2_banks[(t - 1) % 2]
                mm16(pg2, h2T, W2h)
                mm16(pg2, h1T, W2i, stop_last=True)
                h2bm, c2 = epilogue(pg2, c2, "2")
                h2T = transpose_cast(h2bm, "2")
                xa_open(pg2_banks[t % 2])

            if t + 1 < t_steps:
                prime_pg1(t + 1)
            h1T = h1T_new

        # --- flush layer-2 for the final step ---
        pg2 = pg2_banks[(t_steps - 1) % 2]
        mm16(pg2, h2T, W2h)
        mm16(pg2, h1T, W2i, stop_last=True)
        h2bm, c2 = epilogue(pg2, c2, "2")
        h2T = transpose_cast(h2bm, "2")

        # head: y[32,10] = h2 @ W_out.T
        pout = poutp.tile([BC, C], f32, tag="pout")
        for k in range(4):
            nc.tensor.matmul(
                pout[:],
                h2T[:, 32 * k : 32 * k + 32],
                WoT[:, 10 * k : 10 * k + 10],
                start=(k == 0),
                stop=(k == 3),
            )
        ysb = outp.tile([BC, C], f32, tag="ysb")
        nc.vector.tensor_copy(ysb[:], pout[:])
        nc.sync.dma_start(dY, ysb[:])

    nc.compile()
    return nc


def _prep_consts(W_ih1, W_hh1, b_ih1, b_hh1, W_ih2, W_hh2, b_ih2, b_hh2, W_out):
    """Host-side layout transforms shared by all cores."""
    p = _PERM
    w1p = np.asarray(W_hh1, np.float32)[p].copy()  # [2048, 512]
    w2ip = np.asarray(W_ih2, np.float32)[p].copy()
    w2hp = np.asarray(W_hh2, np.float32)[p].copy()

    def wt_img(w):  # -> [128, 4*2048]
        out = np.empty((128, 4 * G4), np.float32)
        for k in range(4):
            out[:, G4 * k : G4 * (k + 1)] = w[:, 128 * k : 128 * (k + 1)].T
        return out

    wih1p = np.asarray(W_ih1, np.float32)[p, 0]
    bias1p = (np.asarray(b_ih1, np.float32) + np.asarray(b_hh1, np.float32))[p]
    bias2p = (np.asarray(b_ih2, np.float32) + np.asarray(b_hh2, np.float32))[p]
    # chunked-layout images for the pg1 DVE prime: [32jh+b, 128gi+h'] = val[512gi+128jh+h']
    def chunk_img(vec):
        out = np.empty((128, 512), np.float32)
        for jh in range(4):
            for gi in range(4):
                out[32 * jh : 32 * jh + 32, 128 * gi : 128 * (gi + 1)] = vec[
                    512 * gi + 128 * jh : 512 * gi + 128 * (jh + 1)
                ][None, :]
        return out

    wi1_img = chunk_img(wih1p)
    b1_img = chunk_img(bias1p)
    # pg2 opener operands: indicator rows x per-jh bias2 image rows
    ind4 = np.zeros((4, 128), np.float32)
    for k in range(4):
        ind4[k, 32 * k : 32 * k + 32] = 1.0
    xb2r = chunk_img(bias2p)[::32][:4].copy()  # [4, 512], row k = jh-chunk k

    wot = np.empty((128, 4 * C), np.float32)
    wo = np.asarray(W_out, np.float32)
    for k in range(4):
        wot[:, C * k : C * (k + 1)] = wo[:, 128 * k : 128 * (k + 1)].T

    import ml_dtypes

    bf = ml_dtypes.bfloat16
    return {
        "w1t": wt_img(w1p).astype(bf),
        "w2it": wt_img(w2ip).astype(bf),
        "w2ht": wt_img(w2hp).astype(bf),
        "wi1img": wi1_img,
        "b1img": b1_img,
        "ind4": ind4.astype(bf),
        "xb2r": xb2r.astype(bf),
        "wot": wot.astype(bf),
        "id32": np.eye(128, dtype=np.float32).astype(bf),
    }


def _prep_xcols(x_shard, t_steps=T):
    """[128, T] fp32: col t = x_t per partition (batch replicated per jh group)."""
    xs = np.asarray(x_shard, np.float32)[:, :t_steps]  # [32, T]
    return np.tile(xs, (4, 1)).reshape(128, t_steps)


def _install_ntff_hook():
    """Provide antenv.axon_hooks (absent in this image) so trace=True works."""
    import sys, types
    if "antenv.axon_hooks" in sys.modules:
        return
    try:
        import antenv
        from trn_agent_boot.trn_boot import _ntff_profile_via_ctypes
    except Exception:
        return
    mod = types.ModuleType("antenv.axon_hooks")
    holder = {}
    mod.set_axon_ntff_profile_hook = lambda h: holder.__setitem__("h", h)
    mod.get_axon_ntff_profile_hook = lambda: holder.get("h")
    sys.modules["antenv.axon_hooks"] = mod
    antenv.axon_hooks = mod
    try:
        hook = _ntff_profile_via_ctypes("/opt/axon/libaxon_pjrt.so")
        if hook is not None:
            mod.set_axon_ntff_profile_hook(hook)
    except Exception:
        pass

def kernel(x, W_ih1, W_hh1, b_ih1, b_hh1, W_ih2, W_hh2, b_ih2, b_hh2, W_out, b_out):
    import sys

    for pth in ("/opt/trn_rl_repo", "/root/.axon_site/_ro/trn_rl_repo"):
        if os.path.isdir(pth) and pth not in sys.path:
            sys.path.append(pth)
    from concourse import bass_utils

    if "nc" not in _BUILD_CACHE:
        _BUILD_CACHE["nc"] = _build(T)
    nc = _BUILD_CACHE["nc"]

    consts = _prep_consts(W_ih1, W_hh1, b_ih1, b_hh1, W_ih2, W_hh2, b_ih2, b_hh2, W_out)
    in_maps = []
    for cidx in range(NCORES):
        sl = slice(BC * cidx, BC * (cidx + 1))
        xs = np.asarray(x)[sl]
        in_maps.append({**consts, "xcols": _prep_xcols(xs)})

    trace = bool(int(os.environ.get("KERNEL_TRACE", "0")))
    if trace:
        _install_ntff_hook()
    res = bass_utils.run_bass_kernel_spmd(nc, in_maps, core_ids=list(range(NCORES)), trace=trace)
    _BUILD_CACHE["last_results"] = res

    out = np.empty((B, C), np.float32)
    bo = np.asarray(b_out, np.float32)
    for cidx in range(NCORES):
        out[BC * cidx : BC * (cidx + 1)] = res.results[cidx]["y"] + bo
    return out


# revision 26
# speedup vs baseline: 1.5995x; 1.3785x over previous
"""Trainium2 Bass kernel: 2-layer LSTM (B=256, T=512, H=512) -> linear head.

Strategy:
  - Data-parallel over batch: 8 cores x 32 rows each, weights replicated,
    no inter-core communication (recurrence is per-batch-row independent).
  - Per step, per layer: gates^T computed batch-major with the h-state as the
    *stationary* matmul operand ([128 h-dims x 32 batch] per K-chunk) and the
    transposed weight matrix W^T as the *moving* operand, 4x col-tiled so all
    four 32-partition output groups of one PSUM bank accumulate concurrently.
  - Gate-dim blocks permuted to (i, f, o, g); the epilogue runs three Act
    instructions sigmoid(i,f) / tanh(g) / sigmoid(o) so the Vector c-update
    chain starts as early as possible; gates are bf16 for 2x DVE throughput.
  - Layer-2 bias injected by a K=2 start=True matmul (rows [x_t; 1] against a
    [0; b2] image) that opens the PSUM accumulation group; layer-1 bias and
    scalar-input term written by a Vector scalar_tensor_tensor prime.
  - One-step software-pipeline skew: iteration t runs L1(t), its epilogue and
    h1-transpose, then L2(t-1) matmuls + epilogue + h2-transpose, so the PE
    stream stays dense and the h1 recurrence cycle never waits on layer 2.
"""

import os
import numpy as np

B, T, H, C = 256, 512, 512, 10
NCORES = 8
BC = B // NCORES  # 32
G4 = 4 * H  # 2048

# gate blocks reordered (i, f, o, g): sigmoids contiguous, tanh block last
_PERM = np.concatenate(
    [np.arange(0, 512), np.arange(512, 1024), np.arange(1536, 2048), np.arange(1024, 1536)]
)

_BUILD_CACHE = {}


def _build(t_steps=T):
    """Trace + schedule + compile the bass module. Returns nc."""
    import concourse.bass as bass
    import concourse.tile as tile
    from concourse import bacc, mybir
    from contextlib import ExitStack

    f32 = mybir.dt.float32
    bf16 = mybir.dt.bfloat16
    AF = mybir.ActivationFunctionType
    assert t_steps % 4 == 0
    tq_len = t_steps // 4  # steps handled per q row-pair

    nc = bacc.Bacc("TRN2", target_bir_lowering=False, debug=False, num_devices=NCORES)

    dW1 = nc.dram_tensor("w1t", (128, 4 * G4), bf16, kind="ExternalInput").ap()
    dW2i = nc.dram_tensor("w2it", (128, 4 * G4), bf16, kind="ExternalInput").ap()
    dW2h = nc.dram_tensor("w2ht", (128, 4 * G4), bf16, kind="ExternalInput").ap()
    dXAUG = nc.dram_tensor("xaug8", (128, 128 * tq_len), bf16, kind="ExternalInput").ap()
    dWB1R = nc.dram_tensor("wb1r", (128, 512), bf16, kind="ExternalInput").ap()
    dIND4 = nc.dram_tensor("ind4", (4, 128), bf16, kind="ExternalInput").ap()
    dXB2R = nc.dram_tensor("xb2r", (4, 512), bf16, kind="ExternalInput").ap()
    dWoT = nc.dram_tensor("wot", (128, 4 * C), bf16, kind="ExternalInput").ap()
    dId = nc.dram_tensor("id32", (128, 128), bf16, kind="ExternalInput").ap()
    dY = nc.dram_tensor("y", (BC, C), f32, kind="ExternalOutput").ap()

    def w_ap(tile_ap, k, jh):
        # cols of W^T image: 2048*k + 512*gi + 128*jh + h' ; returns [128, gi=4, h'=128]
        return tile_ap.rearrange("p (k gi j h) -> p k gi j h", k=4, gi=4, j=4, h=128)[
            :, k, :, jh, :
        ]

    with tile.TileContext(nc) as tc, ExitStack() as ctx:
        const = ctx.enter_context(tc.tile_pool(name="const", bufs=1))
        W1 = const.tile([128, 4 * G4], bf16, tag="w1")
        W2i = const.tile([128, 4 * G4], bf16, tag="w2i")
        W2h = const.tile([128, 4 * G4], bf16, tag="w2h")
        XAUG = const.tile([128, 128 * tq_len], bf16, tag="xaug8")
        WB1R = const.tile([128, 512], bf16, tag="wb1r")
        IND4 = const.tile([4, 128], bf16, tag="ind4")
        XB2R = const.tile([4, 512], bf16, tag="xb2r")
        WoT = const.tile([128, 4 * C], bf16, tag="wot")
        Id32 = const.tile([128, 128], bf16, tag="id32")
        nc.sync.dma_start(W1[:], dW1)
        nc.sync.dma_start(W2i[:], dW2i)
        nc.sync.dma_start(W2h[:], dW2h)
        nc.sync.dma_start(XAUG[:], dXAUG)
        nc.sync.dma_start(WB1R[:], dWB1R)
        nc.sync.dma_start(IND4[:], dIND4)
        nc.sync.dma_start(XB2R[:], dXB2R)
        nc.sync.dma_start(WoT[:], dWoT)
        nc.sync.dma_start(Id32[:], dId)

        pg1p = ctx.enter_context(tc.tile_pool(name="pg1", bufs=1, space="PSUM"))
        pg2p = ctx.enter_context(tc.tile_pool(name="pg2", bufs=1, space="PSUM"))
        ptrp = ctx.enter_context(tc.tile_pool(name="ptr", bufs=2, space="PSUM"))
        poutp = ctx.enter_context(tc.tile_pool(name="pout", bufs=1, space="PSUM"))

        gact = ctx.enter_context(tc.tile_pool(name="gact", bufs=3))
        statep = ctx.enter_context(tc.tile_pool(name="state", bufs=3))
        tmpp = ctx.enter_context(tc.tile_pool(name="tmp", bufs=4))
        outp = ctx.enter_context(tc.tile_pool(name="out", bufs=1))

        h1T = statep.tile([128, 128], bf16, tag="hT1")
        h2T = statep.tile([128, 128], bf16, tag="hT2")
        c1 = statep.tile([128, 128], f32, tag="c1")
        c2 = statep.tile([128, 128], f32, tag="c2")
        for st in (h1T, h2T, c1, c2):
            nc.vector.memset(st[:], 0.0)

        pg1_banks = [pg1p.tile([128, 512], f32, tag="pg1a", name="pg1a"),
                     pg1p.tile([128, 512], f32, tag="pg1b", name="pg1b")]
        pg2_banks = [pg2p.tile([128, 512], f32, tag="pg2a", name="pg2a"),
                     pg2p.tile([128, 512], f32, tag="pg2b", name="pg2b")]

        def prime_pg1(t):
            # opens pg1's group: K=8 matmul of masked-x/indicator rows against
            # [W_ih1; b1] image rows -> pg1 = x_t * W_ih1 + b1, start=True
            q, tqi = divmod(t, tq_len)
            nc.tensor.matmul(
                pg1_banks[t % 2][:],
                XAUG[32 * q : 32 * q + 8, 128 * tqi : 128 * tqi + 128],
                WB1R[32 * q : 32 * q + 8, :],
                start=True, stop=False,
                tile_position=(32 * q, 0),
                skip_group_check=True,
            )

        def mm16(pg, hT, W, stop_last=False):
            for k in range(4):
                for jh in range(4):
                    nc.tensor.matmul(
                        pg[32 * jh : 32 * jh + 32, :],
                        hT[:, 32 * k : 32 * k + 32],
                        w_ap(W[:], k, jh),
                        start=False,
                        stop=(stop_last and k == 3 and jh == 3),
                        tile_position=(0, 32 * jh),
                        skip_group_check=True,
                    )

        def xa_open(pg):
            # opens the pg2 accumulation group with a single K=4 M=128 matmul
            # (indicator rows x bias image) so the whole-bank has_written clear
            # of start=True happens exactly once, then writes b2 everywhere
            nc.tensor.matmul(pg[:], IND4[:], XB2R[:], start=True, stop=False,
                             skip_group_check=True)

        def epilogue(pg, c_prev, tagsuf):
            """sigmoid(i,f,o) / tanh(g) + state update -> (hbm, c_new)."""
            g = gact.tile([128, 512], bf16, tag="g" + tagsuf)
            nc.scalar.activation(g[:, 0:384], pg[:, 0:384], AF.Sigmoid)
            nc.scalar.activation(g[:, 384:512], pg[:, 384:512], AF.Tanh)
            tfc = tmpp.tile([128, 128], f32, tag="tfc" + tagsuf)
            nc.vector.tensor_mul(tfc[:], g[:, 128:256], c_prev[:])
            tig = tmpp.tile([128, 128], bf16, tag="tig" + tagsuf)
            nc.vector.tensor_mul(tig[:], g[:, 0:128], g[:, 384:512])
            c_new = statep.tile([128, 128], f32, tag="c" + tagsuf)
            nc.vector.tensor_add(c_new[:], tig[:], tfc[:])
            tc_ = tmpp.tile([128, 128], bf16, tag="tc" + tagsuf)
            nc.scalar.activation(tc_[:], c_new[:], AF.Tanh)
            hbm = tmpp.tile([128, 128], bf16, tag="hbm" + tagsuf)
            nc.vector.tensor_mul(hbm[:], g[:, 256:384], tc_[:])
            return hbm, c_new

        def transpose_cast(hbm, tagsuf):
            pt = ptrp.tile([128, 128], f32, tag="pt")
            nc.tensor.matmul(pt[:], hbm[:], Id32[:], start=True, stop=True)
            hT_new = statep.tile([128, 128], bf16, tag="hT" + tagsuf)
            nc.vector.tensor_copy(hT_new[:], pt[:])
            return hT_new

        prime_pg1(0)
        h2bm_prev = None
        for t in range(t_steps):
            # PE stream per iteration: L1(t), tr2(t-2), prime(t+1),
            # L2h(t-1), L2i(t-1), tr1(t), xa(t) -- keeps the PE dense while
            # both layers' epilogue chains run on Act/Vector.
            pg1 = pg1_banks[t % 2]
            mm16(pg1, h1T, W1)
            if h2bm_prev is not None:
                h2T = transpose_cast(h2bm_prev, "2")  # tr2 for step t-2
            if t + 1 < t_steps:
                prime_pg1(t + 1)
            h1bm, c1 = epilogue(pg1, c1, "1")
            if t > 0:
                pg2 = pg2_banks[(t - 1) % 2]
                mm16(pg2, h2T, W2h)
                mm16(pg2, h1T, W2i, stop_last=True)
            h1T = transpose_cast(h1bm, "1")
            xa_open(pg2_banks[t % 2])
            if t > 0:
                h2bm_prev, c2 = epilogue(pg2, c2, "2")

        # --- flush layer-2 for the final step ---
        h2T = transpose_cast(h2bm_prev, "2")  # tr2 for step T-2
        pg2 = pg2_banks[(t_steps - 1) % 2]
        mm16(pg2, h2T, W2h)
        mm16(pg2, h1T, W2i, stop_last=True)
        h2bm, c2 = epilogue(pg2, c2, "2")
        h2T = transpose_cast(h2bm, "2")

        # head: y[32,10] = h2 @ W_out.T
        pout = poutp.tile([BC, C], f32, tag="pout")
        for k in range(4):
            nc.tensor.matmul(
                pout[:],
                h2T[:, 32 * k : 32 * k + 32],
                WoT[:, 10 * k : 10 * k + 10],
                start=(k == 0),
                stop=(k == 3),
            )
        ysb = outp.tile([BC, C], f32, tag="ysb")
        nc.vector.tensor_copy(ysb[:], pout[:])
        nc.sync.dma_start(dY, ysb[:])

    nc.compile()
    return nc


def _prep_consts(W_ih1, W_hh1, b_ih1, b_hh1, W_ih2, W_hh2, b_ih2, b_hh2, W_out):
    """Host-side layout transforms shared by all cores."""
    p = _PERM
    w1p = np.asarray(W_hh1, np.float32)[p].copy()  # [2048, 512]
    w2ip = np.asarray(W_ih2, np.float32)[p].copy()
    w2hp = np.asarray(W_hh2, np.float32)[p].copy()

    def wt_img(w):  # -> [128, 4*2048]
        out = np.empty((128, 4 * G4), np.float32)
        for k in range(4):
            out[:, G4 * k : G4 * (k + 1)] = w[:, 128 * k : 128 * (k + 1)].T
        return out

    wih1p = np.asarray(W_ih1, np.float32)[p, 0]
    bias1p = (np.asarray(b_ih1, np.float32) + np.asarray(b_hh1, np.float32))[p]
    bias2p = (np.asarray(b_ih2, np.float32) + np.asarray(b_hh2, np.float32))[p]
    # chunked-layout images for the pg1 DVE prime: [32jh+b, 128gi+h'] = val[512gi+128jh+h']
    def chunk_img(vec):
        out = np.empty((128, 512), np.float32)
        for jh in range(4):
            for gi in range(4):
                out[32 * jh : 32 * jh + 32, 128 * gi : 128 * (gi + 1)] = vec[
                    512 * gi + 128 * jh : 512 * gi + 128 * (jh + 1)
                ][None, :]
        return out

    # pg1 opener rhs: rows 32q+2k = W_ih1 jh-chunk k, rows 32q+2k+1 = b1 chunk k
    wb1r = np.zeros((128, 512), np.float32)
    wi1_rows = chunk_img(wih1p)[::32][:4]  # [4, 512]
    b1_rows = chunk_img(bias1p)[::32][:4]
    for q in range(4):
        for k in range(4):
            wb1r[32 * q + 2 * k] = wi1_rows[k]
            wb1r[32 * q + 2 * k + 1] = b1_rows[k]
    # pg2 opener operands: indicator rows x per-jh bias2 image rows
    ind4 = np.zeros((4, 128), np.float32)
    for k in range(4):
        ind4[k, 32 * k : 32 * k + 32] = 1.0
    xb2r = chunk_img(bias2p)[::32][:4].copy()  # [4, 512], row k = jh-chunk k

    wot = np.empty((128, 4 * C), np.float32)
    wo = np.asarray(W_out, np.float32)
    for k in range(4):
        wot[:, C * k : C * (k + 1)] = wo[:, 128 * k : 128 * (k + 1)].T

    import ml_dtypes

    bf = ml_dtypes.bfloat16
    return {
        "w1t": wt_img(w1p).astype(bf),
        "w2it": wt_img(w2ip).astype(bf),
        "w2ht": wt_img(w2hp).astype(bf),
        "wb1r": wb1r.astype(bf),
        "ind4": ind4.astype(bf),
        "xb2r": xb2r.astype(bf),
        "wot": wot.astype(bf),
        "id32": np.eye(128, dtype=np.float32).astype(bf),
    }


def _prep_xaug8(x_shard, t_steps=T):
    """pg1-opener lhsT image [128, 128*(T/4)]: for quarter q and step t=128q+tqi,
    row 32q+2k holds x[b,t] at col 128*tqi + 32k+b (the jh==k block), row
    32q+2k+1 holds the jh==k indicator."""
    import ml_dtypes

    tq_len = t_steps // 4
    xs = np.asarray(x_shard, np.float32)  # [32, T]
    xa = np.zeros((128, 128 * tq_len), np.float32)
    for q in range(4):
        block = xs[:, q * tq_len : (q + 1) * tq_len].T  # [tqi, b]
        for k in range(4):
            xa[32 * q + 2 * k].reshape(tq_len, 128)[:, 32 * k : 32 * k + 32] = block
            xa[32 * q + 2 * k + 1].reshape(tq_len, 128)[:, 32 * k : 32 * k + 32] = 1.0
    return xa.astype(ml_dtypes.bfloat16)


def _install_ntff_hook():
    """Provide antenv.axon_hooks (absent in this image) so trace=True works."""
    import sys, types
    if "antenv.axon_hooks" in sys.modules:
        return
    try:
        import antenv
        from trn_agent_boot.trn_boot import _ntff_profile_via_ctypes
    except Exception:
        return
    mod = types.ModuleType("antenv.axon_hooks")
    holder = {}
    mod.set_axon_ntff_profile_hook = lambda h: holder.__setitem__("h", h)
    mod.get_axon_ntff_profile_hook = lambda: holder.get("h")
    sys.modules["antenv.axon_hooks"] = mod
    antenv.axon_hooks = mod
    try:
        hook = _ntff_profile_via_ctypes("/opt/axon/libaxon_pjrt.so")
        if hook is not None:
            mod.set_axon_ntff_profile_hook(hook)
    except Exception:
        pass

def kernel(x, W_ih1, W_hh1, b_ih1, b_hh1, W_ih2, W_hh2, b_ih2, b_hh2, W_out, b_out):
    import sys

    for pth in ("/opt/trn_rl_repo", "/root/.axon_site/_ro/trn_rl_repo"):
        if os.path.isdir(pth) and pth not in sys.path:
            sys.path.append(pth)
    from concourse import bass_utils

    if "nc" not in _BUILD_CACHE:
        _BUILD_CACHE["nc"] = _build(T)
    nc = _BUILD_CACHE["nc"]

    consts = _prep_consts(W_ih1, W_hh1, b_ih1, b_hh1, W_ih2, W_hh2, b_ih2, b_hh2, W_out)
    in_maps = []
    for cidx in range(NCORES):
        sl = slice(BC * cidx, BC * (cidx + 1))
        xs = np.asarray(x)[sl]
        in_maps.append({**consts, "xaug8": _prep_xaug8(xs)})

    trace = bool(int(os.environ.get("KERNEL_TRACE", "0")))
    if trace:
        _install_ntff_hook()
    res = bass_utils.run_bass_kernel_spmd(nc, in_maps, core_ids=list(range(NCORES)), trace=trace)
    _BUILD_CACHE["last_results"] = res

    out = np.empty((B, C), np.float32)
    bo = np.asarray(b_out, np.float32)
    for cidx in range(NCORES):
        out[BC * cidx : BC * (cidx + 1)] = res.results[cidx]["y"] + bo
    return out


# revision 27
# speedup vs baseline: 1.8081x; 1.1304x over previous
"""Trainium2 Bass kernel: 2-layer LSTM (B=256, T=512, H=512) -> linear head.

Strategy:
  - Data-parallel over batch: 8 cores x 32 rows each, weights replicated,
    no inter-core communication (recurrence is per-batch-row independent).
  - Per step, per layer: gates^T computed batch-major with the h-state as the
    *stationary* matmul operand ([128 h-dims x 32 batch] per K-chunk) and the
    transposed weight matrix W^T as the *moving* operand, 4x col-tiled so all
    four 32-partition output groups of one PSUM bank accumulate concurrently.
  - Gate-dim blocks permuted to (i, f, o, g); the epilogue runs three Act
    instructions sigmoid(i,f) / tanh(g) / sigmoid(o) so the Vector c-update
    chain starts as early as possible; gates are bf16 for 2x DVE throughput.
  - Layer-2 bias injected by a K=2 start=True matmul (rows [x_t; 1] against a
    [0; b2] image) that opens the PSUM accumulation group; layer-1 bias and
    scalar-input term written by a Vector scalar_tensor_tensor prime.
  - One-step software-pipeline skew: iteration t runs L1(t), its epilogue and
    h1-transpose, then L2(t-1) matmuls + epilogue + h2-transpose, so the PE
    stream stays dense and the h1 recurrence cycle never waits on layer 2.
"""

import os
import numpy as np

B, T, H, C = 256, 512, 512, 10
NCORES = 8
BC = B // NCORES  # 32
G4 = 4 * H  # 2048

# gate blocks reordered (i, f, o, g): sigmoids contiguous, tanh block last
_PERM = np.concatenate(
    [np.arange(0, 512), np.arange(512, 1024), np.arange(1536, 2048), np.arange(1024, 1536)]
)

_BUILD_CACHE = {}


def _build(t_steps=T):
    """Trace + schedule + compile the bass module. Returns nc."""
    import concourse.bass as bass
    import concourse.tile as tile
    from concourse import bacc, mybir
    from contextlib import ExitStack

    f32 = mybir.dt.float32
    bf16 = mybir.dt.bfloat16
    AF = mybir.ActivationFunctionType
    assert t_steps % 4 == 0
    tq_len = t_steps // 4  # steps handled per q row-pair

    nc = bacc.Bacc("TRN2", target_bir_lowering=False, debug=False, num_devices=NCORES)

    dW1 = nc.dram_tensor("w1t", (128, 4 * G4), bf16, kind="ExternalInput").ap()
    dW2i = nc.dram_tensor("w2it", (128, 4 * G4), bf16, kind="ExternalInput").ap()
    dW2h = nc.dram_tensor("w2ht", (128, 4 * G4), bf16, kind="ExternalInput").ap()
    dXAUG = nc.dram_tensor("xaug8", (128, 128 * tq_len), bf16, kind="ExternalInput").ap()
    dWB1R = nc.dram_tensor("wb1r", (128, 512), bf16, kind="ExternalInput").ap()
    dIND4 = nc.dram_tensor("ind4", (4, 128), bf16, kind="ExternalInput").ap()
    dXB2R = nc.dram_tensor("xb2r", (4, 512), bf16, kind="ExternalInput").ap()
    dWoT = nc.dram_tensor("wot", (128, 4 * C), bf16, kind="ExternalInput").ap()
    dId = nc.dram_tensor("id32", (128, 128), bf16, kind="ExternalInput").ap()
    dY = nc.dram_tensor("y", (BC, C), f32, kind="ExternalOutput").ap()

    def w_ap(tile_ap, k, jh):
        # cols of W^T image: 2048*k + 512*gi + 128*jh + h' ; returns [128, gi=4, h'=128]
        return tile_ap.rearrange("p (k gi j h) -> p k gi j h", k=4, gi=4, j=4, h=128)[
            :, k, :, jh, :
        ]

    with tile.TileContext(nc) as tc, ExitStack() as ctx:
        const = ctx.enter_context(tc.tile_pool(name="const", bufs=1))
        W1 = const.tile([128, 4 * G4], bf16, tag="w1")
        W2i = const.tile([128, 4 * G4], bf16, tag="w2i")
        W2h = const.tile([128, 4 * G4], bf16, tag="w2h")
        XAUG = const.tile([128, 128 * tq_len], bf16, tag="xaug8")
        WB1R = const.tile([128, 512], bf16, tag="wb1r")
        IND4 = const.tile([4, 128], bf16, tag="ind4")
        XB2R = const.tile([4, 512], bf16, tag="xb2r")
        WoT = const.tile([128, 4 * C], bf16, tag="wot")
        Id32 = const.tile([128, 128], bf16, tag="id32")
        nc.sync.dma_start(W1[:], dW1)
        nc.sync.dma_start(W2i[:], dW2i)
        nc.sync.dma_start(W2h[:], dW2h)
        nc.sync.dma_start(XAUG[:], dXAUG)
        nc.sync.dma_start(WB1R[:], dWB1R)
        nc.sync.dma_start(IND4[:], dIND4)
        nc.sync.dma_start(XB2R[:], dXB2R)
        nc.sync.dma_start(WoT[:], dWoT)
        nc.sync.dma_start(Id32[:], dId)

        pg1p = ctx.enter_context(tc.tile_pool(name="pg1", bufs=1, space="PSUM"))
        pg2p = ctx.enter_context(tc.tile_pool(name="pg2", bufs=1, space="PSUM"))
        ptrp = ctx.enter_context(tc.tile_pool(name="ptr", bufs=2, space="PSUM"))
        poutp = ctx.enter_context(tc.tile_pool(name="pout", bufs=1, space="PSUM"))

        gact = ctx.enter_context(tc.tile_pool(name="gact", bufs=3))
        statep = ctx.enter_context(tc.tile_pool(name="state", bufs=3))
        tmpp = ctx.enter_context(tc.tile_pool(name="tmp", bufs=4))
        outp = ctx.enter_context(tc.tile_pool(name="out", bufs=1))

        h1T = statep.tile([128, 128], bf16, tag="hT1")
        h2T = statep.tile([128, 128], bf16, tag="hT2")
        c1 = statep.tile([128, 128], f32, tag="c1")
        c2 = statep.tile([128, 128], f32, tag="c2")
        for st in (h1T, h2T, c1, c2):
            nc.vector.memset(st[:], 0.0)

        pg1_banks = [pg1p.tile([128, 512], f32, tag="pg1a", name="pg1a"),
                     pg1p.tile([128, 512], f32, tag="pg1b", name="pg1b")]
        pg2_banks = [pg2p.tile([128, 512], f32, tag="pg2a", name="pg2a"),
                     pg2p.tile([128, 512], f32, tag="pg2b", name="pg2b")]

        def prime_pg1(t):
            # opens pg1's group: K=8 matmul of masked-x/indicator rows against
            # [W_ih1; b1] image rows -> pg1 = x_t * W_ih1 + b1, start=True
            q, tqi = divmod(t, tq_len)
            nc.tensor.matmul(
                pg1_banks[t % 2][:],
                XAUG[32 * q : 32 * q + 8, 128 * tqi : 128 * tqi + 128],
                WB1R[32 * q : 32 * q + 8, :],
                start=True, stop=False,
                tile_position=(32 * q, 0),
                skip_group_check=True,
            )

        def mm16(pg, hT, W, stop_last=False):
            for k in range(4):
                for jh in range(4):
                    nc.tensor.matmul(
                        pg[32 * jh : 32 * jh + 32, :],
                        hT[:, 32 * k : 32 * k + 32],
                        w_ap(W[:], k, jh),
                        start=False,
                        stop=(stop_last and k == 3 and jh == 3),
                        tile_position=(0, 32 * jh),
                        skip_group_check=True,
                    )

        def xa_open(pg):
            # opens the pg2 accumulation group with a single K=4 M=128 matmul
            # (indicator rows x bias image) so the whole-bank has_written clear
            # of start=True happens exactly once, then writes b2 everywhere
            nc.tensor.matmul(pg[:], IND4[:], XB2R[:], start=True, stop=False,
                             skip_group_check=True)

        def epilogue(pg, c_prev, tagsuf):
            """sigmoid(i,f,o) / tanh(g) + state update -> (hbm, c_new)."""
            g = gact.tile([128, 512], bf16, tag="g" + tagsuf)
            nc.scalar.activation(g[:, 0:384], pg[:, 0:384], AF.Sigmoid)
            nc.scalar.activation(g[:, 384:512], pg[:, 384:512], AF.Tanh)
            tfc = tmpp.tile([128, 128], f32, tag="tfc" + tagsuf)
            nc.vector.tensor_mul(tfc[:], g[:, 128:256], c_prev[:])
            tig = tmpp.tile([128, 128], bf16, tag="tig" + tagsuf)
            nc.vector.tensor_mul(tig[:], g[:, 0:128], g[:, 384:512])
            c_new = statep.tile([128, 128], f32, tag="c" + tagsuf)
            nc.vector.tensor_add(c_new[:], tig[:], tfc[:])
            tc_ = tmpp.tile([128, 128], bf16, tag="tc" + tagsuf)
            nc.scalar.activation(tc_[:], c_new[:], AF.Tanh)
            hbm = tmpp.tile([128, 128], bf16, tag="hbm" + tagsuf)
            nc.vector.tensor_mul(hbm[:], g[:, 256:384], tc_[:])
            return hbm, c_new

        def transpose_cast(hbm, tagsuf):
            pt = ptrp.tile([128, 128], f32, tag="pt")
            nc.tensor.matmul(pt[:], hbm[:], Id32[:], start=True, stop=True)
            hT_new = statep.tile([128, 128], bf16, tag="hT" + tagsuf)
            nc.vector.tensor_copy(hT_new[:], pt[:])
            return hT_new

        prime_pg1(0)
        h2bm_prev = None
        for t in range(t_steps):
            # PE stream per iteration: L1(t), prime(t+1), tr2(t-2), L2i(t-1),
            # L2h(t-1), tr1(t), xa(t).  L2i only needs h1T(t-1) (ready at
            # iteration start) so it fills the PE idle during the L1 epilogue;
            # L2h needs the late h2T(t-2) and closes the pg2 group.
            pg1 = pg1_banks[t % 2]
            mm16(pg1, h1T, W1)
            if t + 1 < t_steps:
                prime_pg1(t + 1)
            if h2bm_prev is not None:
                h2T = transpose_cast(h2bm_prev, "2")  # tr2 for step t-2
            h1bm, c1 = epilogue(pg1, c1, "1")
            if t > 0:
                pg2 = pg2_banks[(t - 1) % 2]
                mm16(pg2, h1T, W2i)
                mm16(pg2, h2T, W2h, stop_last=True)
            h1T = transpose_cast(h1bm, "1")
            xa_open(pg2_banks[t % 2])
            if t > 0:
                h2bm_prev, c2 = epilogue(pg2, c2, "2")

        # --- flush layer-2 for the final step ---
        h2T = transpose_cast(h2bm_prev, "2")  # tr2 for step T-2
        pg2 = pg2_banks[(t_steps - 1) % 2]
        mm16(pg2, h1T, W2i)
        mm16(pg2, h2T, W2h, stop_last=True)
        h2bm, c2 = epilogue(pg2, c2, "2")
        h2T = transpose_cast(h2bm, "2")

        # head: y[32,10] = h2 @ W_out.T
        pout = poutp.tile([BC, C], f32, tag="pout")
        for k in range(4):
            nc.tensor.matmul(
                pout[:],
                h2T[:, 32 * k : 32 * k + 32],
                WoT[:, 10 * k : 10 * k + 10],
                start=(k == 0),
                stop=(k == 3),
            )
        ysb = outp.tile([BC, C], f32, tag="ysb")
        nc.vector.tensor_copy(ysb[:], pout[:])
        nc.sync.dma_start(dY, ysb[:])

    nc.compile()
    return nc


def _prep_consts(W_ih1, W_hh1, b_ih1, b_hh1, W_ih2, W_hh2, b_ih2, b_hh2, W_out):
    """Host-side layout transforms shared by all cores."""
    p = _PERM
    w1p = np.asarray(W_hh1, np.float32)[p].copy()  # [2048, 512]
    w2ip = np.asarray(W_ih2, np.float32)[p].copy()
    w2hp = np.asarray(W_hh2, np.float32)[p].copy()

    def wt_img(w):  # -> [128, 4*2048]
        out = np.empty((128, 4 * G4), np.float32)
        for k in range(4):
            out[:, G4 * k : G4 * (k + 1)] = w[:, 128 * k : 128 * (k + 1)].T
        return out

    wih1p = np.asarray(W_ih1, np.float32)[p, 0]
    bias1p = (np.asarray(b_ih1, np.float32) + np.asarray(b_hh1, np.float32))[p]
    bias2p = (np.asarray(b_ih2, np.float32) + np.asarray(b_hh2, np.float32))[p]
    # chunked-layout images for the pg1 DVE prime: [32jh+b, 128gi+h'] = val[512gi+128jh+h']
    def chunk_img(vec):
        out = np.empty((128, 512), np.float32)
        for jh in range(4):
            for gi in range(4):
                out[32 * jh : 32 * jh + 32, 128 * gi : 128 * (gi + 1)] = vec[
                    512 * gi + 128 * jh : 512 * gi + 128 * (jh + 1)
                ][None, :]
        return out

    # pg1 opener rhs: rows 32q+2k = W_ih1 jh-chunk k, rows 32q+2k+1 = b1 chunk k
    wb1r = np.zeros((128, 512), np.float32)
    wi1_rows = chunk_img(wih1p)[::32][:4]  # [4, 512]
    b1_rows = chunk_img(bias1p)[::32][:4]
    for q in range(4):
        for k in range(4):
            wb1r[32 * q + 2 * k] = wi1_rows[k]
            wb1r[32 * q + 2 * k + 1] = b1_rows[k]
    # pg2 opener operands: indicator rows x per-jh bias2 image rows
    ind4 = np.zeros((4, 128), np.float32)
    for k in range(4):
        ind4[k, 32 * k : 32 * k + 32] = 1.0
    xb2r = chunk_img(bias2p)[::32][:4].copy()  # [4, 512], row k = jh-chunk k

    wot = np.empty((128, 4 * C), np.float32)
    wo = np.asarray(W_out, np.float32)
    for k in range(4):
        wot[:, C * k : C * (k + 1)] = wo[:, 128 * k : 128 * (k + 1)].T

    import ml_dtypes

    bf = ml_dtypes.bfloat16
    return {
        "w1t": wt_img(w1p).astype(bf),
        "w2it": wt_img(w2ip).astype(bf),
        "w2ht": wt_img(w2hp).astype(bf),
        "wb1r": wb1r.astype(bf),
        "ind4": ind4.astype(bf),
        "xb2r": xb2r.astype(bf),
        "wot": wot.astype(bf),
        "id32": np.eye(128, dtype=np.float32).astype(bf),
    }


def _prep_xaug8(x_shard, t_steps=T):
    """pg1-opener lhsT image [128, 128*(T/4)]: for quarter q and step t=128q+tqi,
    row 32q+2k holds x[b,t] at col 128*tqi + 32k+b (the jh==k block), row
    32q+2k+1 holds the jh==k indicator."""
    import ml_dtypes

    tq_len = t_steps // 4
    xs = np.asarray(x_shard, np.float32)  # [32, T]
    xa = np.zeros((128, 128 * tq_len), np.float32)
    for q in range(4):
        block = xs[:, q * tq_len : (q + 1) * tq_len].T  # [tqi, b]
        for k in range(4):
            xa[32 * q + 2 * k].reshape(tq_len, 128)[:, 32 * k : 32 * k + 32] = block
            xa[32 * q + 2 * k + 1].reshape(tq_len, 128)[:, 32 * k : 32 * k + 32] = 1.0
    return xa.astype(ml_dtypes.bfloat16)


def _install_ntff_hook():
    """Provide antenv.axon_hooks (absent in this image) so trace=True works."""
    import sys, types
    if "antenv.axon_hooks" in sys.modules:
        return
    try:
        import antenv
        from trn_agent_boot.trn_boot import _ntff_profile_via_ctypes
    except Exception:
        return
    mod = types.ModuleType("antenv.axon_hooks")
    holder = {}
    mod.set_axon_ntff_profile_hook = lambda h: holder.__setitem__("h", h)
    mod.get_axon_ntff_profile_hook = lambda: holder.get("h")
    sys.modules["antenv.axon_hooks"] = mod
    antenv.axon_hooks = mod
    try:
        hook = _ntff_profile_via_ctypes("/opt/axon/libaxon_pjrt.so")
        if hook is not None:
            mod.set_axon_ntff_profile_hook(hook)
    except Exception:
        pass

def kernel(x, W_ih1, W_hh1, b_ih1, b_hh1, W_ih2, W_hh2, b_ih2, b_hh2, W_out, b_out):
    import sys

    for pth in ("/opt/trn_rl_repo", "/root/.axon_site/_ro/trn_rl_repo"):
        if os.path.isdir(pth) and pth not in sys.path:
            sys.path.append(pth)
    from concourse import bass_utils

    if "nc" not in _BUILD_CACHE:
        _BUILD_CACHE["nc"] = _build(T)
    nc = _BUILD_CACHE["nc"]

    consts = _prep_consts(W_ih1, W_hh1, b_ih1, b_hh1, W_ih2, W_hh2, b_ih2, b_hh2, W_out)
    in_maps = []
    for cidx in range(NCORES):
        sl = slice(BC * cidx, BC * (cidx + 1))
        xs = np.asarray(x)[sl]
        in_maps.append({**consts, "xaug8": _prep_xaug8(xs)})

    trace = bool(int(os.environ.get("KERNEL_TRACE", "0")))
    if trace:
        _install_ntff_hook()
    res = bass_utils.run_bass_kernel_spmd(nc, in_maps, core_ids=list(range(NCORES)), trace=trace)
    _BUILD_CACHE["last_results"] = res

    out = np.empty((B, C), np.float32)
    bo = np.asarray(b_out, np.float32)
    for cidx in range(NCORES):
        out[BC * cidx : BC * (cidx + 1)] = res.results[cidx]["y"] + bo
    return out


# revision 28
# speedup vs baseline: 1.8150x; 1.0038x over previous
"""Trainium2 Bass kernel: 2-layer LSTM (B=256, T=512, H=512) -> linear head.

Strategy:
  - Data-parallel over batch: 8 cores x 32 rows each, weights replicated,
    no inter-core communication (recurrence is per-batch-row independent).
  - Per step, per layer: gates^T computed batch-major with the h-state as the
    *stationary* matmul operand ([128 h-dims x 32 batch] per K-chunk) and the
    transposed weight matrix W^T as the *moving* operand, 4x col-tiled so all
    four 32-partition output groups of one PSUM bank accumulate concurrently.
  - Gate-dim blocks permuted to (i, f, o, g); the epilogue runs three Act
    instructions sigmoid(i,f) / tanh(g) / sigmoid(o) so the Vector c-update
    chain starts as early as possible; gates are bf16 for 2x DVE throughput.
  - Layer-2 bias injected by a K=2 start=True matmul (rows [x_t; 1] against a
    [0; b2] image) that opens the PSUM accumulation group; layer-1 bias and
    scalar-input term written by a Vector scalar_tensor_tensor prime.
  - One-step software-pipeline skew: iteration t runs L1(t), its epilogue and
    h1-transpose, then L2(t-1) matmuls + epilogue + h2-transpose, so the PE
    stream stays dense and the h1 recurrence cycle never waits on layer 2.
"""

import os
import numpy as np

B, T, H, C = 256, 512, 512, 10
NCORES = 8
BC = B // NCORES  # 32
G4 = 4 * H  # 2048

# gate blocks reordered (i, f, o, g): sigmoids contiguous, tanh block last
_PERM = np.concatenate(
    [np.arange(0, 512), np.arange(512, 1024), np.arange(1536, 2048), np.arange(1024, 1536)]
)

_BUILD_CACHE = {}


def _build(t_steps=T):
    """Trace + schedule + compile the bass module. Returns nc."""
    import concourse.bass as bass
    import concourse.tile as tile
    from concourse import bacc, mybir
    from contextlib import ExitStack

    f32 = mybir.dt.float32
    bf16 = mybir.dt.bfloat16
    AF = mybir.ActivationFunctionType
    assert t_steps % 4 == 0
    tq_len = t_steps // 4  # steps handled per q row-pair

    nc = bacc.Bacc("TRN2", target_bir_lowering=False, debug=False, num_devices=NCORES)

    dW1 = nc.dram_tensor("w1t", (128, 4 * G4), bf16, kind="ExternalInput").ap()
    dW2i = nc.dram_tensor("w2it", (128, 4 * G4), bf16, kind="ExternalInput").ap()
    dW2h = nc.dram_tensor("w2ht", (128, 4 * G4), bf16, kind="ExternalInput").ap()
    dXAUG = nc.dram_tensor("xaug8", (128, 128 * tq_len), bf16, kind="ExternalInput").ap()
    dWB1R = nc.dram_tensor("wb1r", (128, 512), bf16, kind="ExternalInput").ap()
    dIND4 = nc.dram_tensor("ind4", (4, 128), bf16, kind="ExternalInput").ap()
    dXB2R = nc.dram_tensor("xb2r", (4, 512), bf16, kind="ExternalInput").ap()
    dWoT = nc.dram_tensor("wot", (128, 4 * C), bf16, kind="ExternalInput").ap()
    dId = nc.dram_tensor("id32", (128, 128), bf16, kind="ExternalInput").ap()
    dY = nc.dram_tensor("y", (BC, C), f32, kind="ExternalOutput").ap()

    def w_ap(tile_ap, k, jh):
        # cols of W^T image: 2048*k + 512*gi + 128*jh + h' ; returns [128, gi=4, h'=128]
        return tile_ap.rearrange("p (k gi j h) -> p k gi j h", k=4, gi=4, j=4, h=128)[
            :, k, :, jh, :
        ]

    with tile.TileContext(nc) as tc, ExitStack() as ctx:
        const = ctx.enter_context(tc.tile_pool(name="const", bufs=1))
        W1 = const.tile([128, 4 * G4], bf16, tag="w1")
        W2i = const.tile([128, 4 * G4], bf16, tag="w2i")
        W2h = const.tile([128, 4 * G4], bf16, tag="w2h")
        XAUG = const.tile([128, 128 * tq_len], bf16, tag="xaug8")
        WB1R = const.tile([128, 512], bf16, tag="wb1r")
        IND4 = const.tile([4, 128], bf16, tag="ind4")
        XB2R = const.tile([4, 512], bf16, tag="xb2r")
        WoT = const.tile([128, 4 * C], bf16, tag="wot")
        Id32 = const.tile([128, 128], bf16, tag="id32")
        nc.sync.dma_start(W1[:], dW1)
        nc.sync.dma_start(W2i[:], dW2i)
        nc.sync.dma_start(W2h[:], dW2h)
        nc.sync.dma_start(XAUG[:], dXAUG)
        nc.sync.dma_start(WB1R[:], dWB1R)
        nc.sync.dma_start(IND4[:], dIND4)
        nc.sync.dma_start(XB2R[:], dXB2R)
        nc.sync.dma_start(WoT[:], dWoT)
        nc.sync.dma_start(Id32[:], dId)

        pg1p = ctx.enter_context(tc.tile_pool(name="pg1", bufs=1, space="PSUM"))
        pg2p = ctx.enter_context(tc.tile_pool(name="pg2", bufs=1, space="PSUM"))
        ptrp = ctx.enter_context(tc.tile_pool(name="ptr", bufs=2, space="PSUM"))
        poutp = ctx.enter_context(tc.tile_pool(name="pout", bufs=1, space="PSUM"))

        gact = ctx.enter_context(tc.tile_pool(name="gact", bufs=3))
        statep = ctx.enter_context(tc.tile_pool(name="state", bufs=3))
        tmpp = ctx.enter_context(tc.tile_pool(name="tmp", bufs=4))
        outp = ctx.enter_context(tc.tile_pool(name="out", bufs=1))

        h1T = statep.tile([128, 128], bf16, tag="hT1")
        h2T = statep.tile([128, 128], bf16, tag="hT2")
        c1 = statep.tile([128, 128], f32, tag="c1")
        c2 = statep.tile([128, 128], f32, tag="c2")
        for st in (h1T, h2T, c1, c2):
            nc.vector.memset(st[:], 0.0)

        pg1_banks = [pg1p.tile([128, 512], f32, tag="pg1a", name="pg1a"),
                     pg1p.tile([128, 512], f32, tag="pg1b", name="pg1b")]
        pg2_banks = [pg2p.tile([128, 512], f32, tag="pg2a", name="pg2a"),
                     pg2p.tile([128, 512], f32, tag="pg2b", name="pg2b")]

        def prime_pg1(t):
            # opens pg1's group: K=8 matmul of masked-x/indicator rows against
            # [W_ih1; b1] image rows -> pg1 = x_t * W_ih1 + b1, start=True
            q, tqi = divmod(t, tq_len)
            nc.tensor.matmul(
                pg1_banks[t % 2][:],
                XAUG[32 * q : 32 * q + 8, 128 * tqi : 128 * tqi + 128],
                WB1R[32 * q : 32 * q + 8, :],
                start=True, stop=False,
                tile_position=(32 * q, 0),
                skip_group_check=True,
            )

        def mm16(pg, hT, W, stop_last=False):
            for k in range(4):
                for jh in range(4):
                    nc.tensor.matmul(
                        pg[32 * jh : 32 * jh + 32, :],
                        hT[:, 32 * k : 32 * k + 32],
                        w_ap(W[:], k, jh),
                        start=False,
                        stop=(stop_last and k == 3 and jh == 3),
                        tile_position=(0, 32 * jh),
                        skip_group_check=True,
                    )

        def xa_open(pg):
            # opens the pg2 accumulation group with a single K=4 M=128 matmul
            # (indicator rows x bias image) so the whole-bank has_written clear
            # of start=True happens exactly once, then writes b2 everywhere
            nc.tensor.matmul(pg[:], IND4[:], XB2R[:], start=True, stop=False,
                             skip_group_check=True)

        def epilogue(pg, c_prev, tagsuf):
            """sigmoid(i,f,o) / tanh(g) + state update -> (hbm, c_new)."""
            g = gact.tile([128, 512], bf16, tag="g" + tagsuf)
            nc.scalar.activation(g[:, 0:384], pg[:, 0:384], AF.Sigmoid)
            nc.scalar.activation(g[:, 384:512], pg[:, 384:512], AF.Tanh)
            tfc = tmpp.tile([128, 128], f32, tag="tfc" + tagsuf)
            nc.vector.tensor_mul(tfc[:], g[:, 128:256], c_prev[:])
            tig = tmpp.tile([128, 128], bf16, tag="tig" + tagsuf)
            nc.vector.tensor_mul(tig[:], g[:, 0:128], g[:, 384:512])
            c_new = statep.tile([128, 128], f32, tag="c" + tagsuf)
            nc.vector.tensor_add(c_new[:], tig[:], tfc[:])
            tc_ = tmpp.tile([128, 128], bf16, tag="tc" + tagsuf)
            nc.scalar.activation(tc_[:], c_new[:], AF.Tanh)
            hbm = tmpp.tile([128, 128], bf16, tag="hbm" + tagsuf)
            nc.vector.tensor_mul(hbm[:], g[:, 256:384], tc_[:])
            return hbm, c_new

        def transpose_cast(hbm, tagsuf):
            pt = ptrp.tile([128, 128], f32, tag="pt")
            nc.tensor.matmul(pt[:], hbm[:], Id32[:], start=True, stop=True)
            hT_new = statep.tile([128, 128], bf16, tag="hT" + tagsuf)
            # split the PSUM->SBUF cast so the k-chunk matmuls reading
            # hT[:, 0:64] can start while the second half is still copying
            nc.vector.tensor_copy(hT_new[:, 0:64], pt[:, 0:64])
            nc.vector.tensor_copy(hT_new[:, 64:128], pt[:, 64:128])
            return hT_new

        prime_pg1(0)
        h2bm_prev = None
        for t in range(t_steps):
            # PE stream per iteration: L1(t), prime(t+1), tr2(t-2), L2i(t-1),
            # L2h(t-1), tr1(t), xa(t).  L2i only needs h1T(t-1) (ready at
            # iteration start) so it fills the PE idle during the L1 epilogue;
            # L2h needs the late h2T(t-2) and closes the pg2 group.
            pg1 = pg1_banks[t % 2]
            mm16(pg1, h1T, W1)
            if t + 1 < t_steps:
                prime_pg1(t + 1)
            if h2bm_prev is not None:
                h2T = transpose_cast(h2bm_prev, "2")  # tr2 for step t-2
            h1bm, c1 = epilogue(pg1, c1, "1")
            if t > 0:
                pg2 = pg2_banks[(t - 1) % 2]
                mm16(pg2, h1T, W2i)
                mm16(pg2, h2T, W2h, stop_last=True)
            h1T = transpose_cast(h1bm, "1")
            xa_open(pg2_banks[t % 2])
            if t > 0:
                h2bm_prev, c2 = epilogue(pg2, c2, "2")

        # --- flush layer-2 for the final step ---
        h2T = transpose_cast(h2bm_prev, "2")  # tr2 for step T-2
        pg2 = pg2_banks[(t_steps - 1) % 2]
        mm16(pg2, h1T, W2i)
        mm16(pg2, h2T, W2h, stop_last=True)
        h2bm, c2 = epilogue(pg2, c2, "2")
        h2T = transpose_cast(h2bm, "2")

        # head: y[32,10] = h2 @ W_out.T
        pout = poutp.tile([BC, C], f32, tag="pout")
        for k in range(4):
            nc.tensor.matmul(
                pout[:],
                h2T[:, 32 * k : 32 * k + 32],
                WoT[:, 10 * k : 10 * k + 10],
                start=(k == 0),
                stop=(k == 3),
            )
        ysb = outp.tile([BC, C], f32, tag="ysb")
        nc.vector.tensor_copy(ysb[:], pout[:])
        nc.sync.dma_start(dY, ysb[:])

    nc.compile()
    return nc


def _prep_consts(W_ih1, W_hh1, b_ih1, b_hh1, W_ih2, W_hh2, b_ih2, b_hh2, W_out):
    """Host-side layout transforms shared by all cores."""
    p = _PERM
    w1p = np.asarray(W_hh1, np.float32)[p].copy()  # [2048, 512]
    w2ip = np.asarray(W_ih2, np.float32)[p].copy()
    w2hp = np.asarray(W_hh2, np.float32)[p].copy()

    def wt_img(w):  # -> [128, 4*2048]
        out = np.empty((128, 4 * G4), np.float32)
        for k in range(4):
            out[:, G4 * k : G4 * (k + 1)] = w[:, 128 * k : 128 * (k + 1)].T
        return out

    wih1p = np.asarray(W_ih1, np.float32)[p, 0]
    bias1p = (np.asarray(b_ih1, np.float32) + np.asarray(b_hh1, np.float32))[p]
    bias2p = (np.asarray(b_ih2, np.float32) + np.asarray(b_hh2, np.float32))[p]
    # chunked-layout images for the pg1 DVE prime: [32jh+b, 128gi+h'] = val[512gi+128jh+h']
    def chunk_img(vec):
        out = np.empty((128, 512), np.float32)
        for jh in range(4):
            for gi in range(4):
                out[32 * jh : 32 * jh + 32, 128 * gi : 128 * (gi + 1)] = vec[
                    512 * gi + 128 * jh : 512 * gi + 128 * (jh + 1)
                ][None, :]
        return out

    # pg1 opener rhs: rows 32q+2k = W_ih1 jh-chunk k, rows 32q+2k+1 = b1 chunk k
    wb1r = np.zeros((128, 512), np.float32)
    wi1_rows = chunk_img(wih1p)[::32][:4]  # [4, 512]
    b1_rows = chunk_img(bias1p)[::32][:4]
    for q in range(4):
        for k in range(4):
            wb1r[32 * q + 2 * k] = wi1_rows[k]
            wb1r[32 * q + 2 * k + 1] = b1_rows[k]
    # pg2 opener operands: indicator rows x per-jh bias2 image rows
    ind4 = np.zeros((4, 128), np.float32)
    for k in range(4):
        ind4[k, 32 * k : 32 * k + 32] = 1.0
    xb2r = chunk_img(bias2p)[::32][:4].copy()  # [4, 512], row k = jh-chunk k

    wot = np.empty((128, 4 * C), np.float32)
    wo = np.asarray(W_out, np.float32)
    for k in range(4):
        wot[:, C * k : C * (k + 1)] = wo[:, 128 * k : 128 * (k + 1)].T

    import ml_dtypes

    bf = ml_dtypes.bfloat16
    return {
        "w1t": wt_img(w1p).astype(bf),
        "w2it": wt_img(w2ip).astype(bf),
        "w2ht": wt_img(w2hp).astype(bf),
        "wb1r": wb1r.astype(bf),
        "ind4": ind4.astype(bf),
        "xb2r": xb2r.astype(bf),
        "wot": wot.astype(bf),
        "id32": np.eye(128, dtype=np.float32).astype(bf),
    }


def _prep_xaug8(x_shard, t_steps=T):
    """pg1-opener lhsT image [128, 128*(T/4)]: for quarter q and step t=128q+tqi,
    row 32q+2k holds x[b,t] at col 128*tqi + 32k+b (the jh==k block), row
    32q+2k+1 holds the jh==k indicator."""
    import ml_dtypes

    tq_len = t_steps // 4
    xs = np.asarray(x_shard, np.float32)  # [32, T]
    xa = np.zeros((128, 128 * tq_len), np.float32)
    for q in range(4):
        block = xs[:, q * tq_len : (q + 1) * tq_len].T  # [tqi, b]
        for k in range(4):
            xa[32 * q + 2 * k].reshape(tq_len, 128)[:, 32 * k : 32 * k + 32] = block
            xa[32 * q + 2 * k + 1].reshape(tq_len, 128)[:, 32 * k : 32 * k + 32] = 1.0
    return xa.astype(ml_dtypes.bfloat16)


def _install_ntff_hook():
    """Provide antenv.axon_hooks (absent in this image) so trace=True works."""
    import sys, types
    if "antenv.axon_hooks" in sys.modules:
        return
    try:
        import antenv
        from trn_agent_boot.trn_boot import _ntff_profile_via_ctypes
    except Exception:
        return
    mod = types.ModuleType("antenv.axon_hooks")
    holder = {}
    mod.set_axon_ntff_profile_hook = lambda h: holder.__setitem__("h", h)
    mod.get_axon_ntff_profile_hook = lambda: holder.get("h")
    sys.modules["antenv.axon_hooks"] = mod
    antenv.axon_hooks = mod
    try:
        hook = _ntff_profile_via_ctypes("/opt/axon/libaxon_pjrt.so")
        if hook is not None:
            mod.set_axon_ntff_profile_hook(hook)
    except Exception:
        pass

def kernel(x, W_ih1, W_hh1, b_ih1, b_hh1, W_ih2, W_hh2, b_ih2, b_hh2, W_out, b_out):
    import sys

    for pth in ("/opt/trn_rl_repo", "/root/.axon_site/_ro/trn_rl_repo"):
        if os.path.isdir(pth) and pth not in sys.path:
            sys.path.append(pth)
    from concourse import bass_utils

    if "nc" not in _BUILD_CACHE:
        _BUILD_CACHE["nc"] = _build(T)
    nc = _BUILD_CACHE["nc"]

    consts = _prep_consts(W_ih1, W_hh1, b_ih1, b_hh1, W_ih2, W_hh2, b_ih2, b_hh2, W_out)
    in_maps = []
    for cidx in range(NCORES):
        sl = slice(BC * cidx, BC * (cidx + 1))
        xs = np.asarray(x)[sl]
        in_maps.append({**consts, "xaug8": _prep_xaug8(xs)})

    trace = bool(int(os.environ.get("KERNEL_TRACE", "0")))
    if trace:
        _install_ntff_hook()
    res = bass_utils.run_bass_kernel_spmd(nc, in_maps, core_ids=list(range(NCORES)), trace=trace)
    _BUILD_CACHE["last_results"] = res

    out = np.empty((B, C), np.float32)
    bo = np.asarray(b_out, np.float32)
    for cidx in range(NCORES):
        out[BC * cidx : BC * (cidx + 1)] = res.results[cidx]["y"] + bo
    return out
